# revision 19
# baseline (speedup 1.0000x reference)
"""Grouped GEMM (MoE block-diagonal) on 8 Trainium2 NeuronCores.

Problem: x [262144, 256] bf16, w [1024, 256] bf16 (G=8 experts of [128, 256]).
Rows g*32768:(g+1)*32768 of x belong to expert g.
Output [262144, 1024] bf16, block-diagonal: out[rows_g, g*128:(g+1)*128] = x_g @ w_g^T.

Strategy (expert-parallel, fully SBUF-resident):
  - Core g gets expert g: x_g [32768, 256] and w_g [128, 256].
  - Host pre-transposes both operands so the contraction dim K lands on SBUF
    partitions (PE matmul contracts over the partition dim).
  - x (128 KiB/partition) and y (64 KiB/partition) both live in SBUF for the
    whole kernel: every x-load DMA is issued up-front with no dependencies, so
    the load stream runs at the HBM roofline and HBM-contention jitter cannot
    cascade through buffer-recycle chains.
  - Chunk sizes taper: small first chunk so compute starts early, small last
    chunks so the compute->copy->store drain after the final load is short.
  - Matmuls are grouped per 2048-token super-tile as 4x(w0) then 4x(w1), so
    LDWEIGHTS fires once per 4 matmuls and the PE stays densely busy (keeps
    the HAM clock at 2.4 GHz instead of throttled 1.2 GHz).
  - PSUM->SBUF cast copies alternate DVE/ACT; y stores ride the ACT HWDGE
    ring while x loads own the SP ring.
"""

import sys

for _p in ("/opt/trn_rl_repo", "/root/.axon_site/_ro/trn_rl_repo"):
    if _p not in sys.path:
        sys.path.insert(0, _p)

import numpy as np

G = 8          # experts == cores
K = 256        # contraction dim
N = 128        # output dim per expert
M = 262144     # total tokens
MPC = M // G   # tokens per core = 32768

PT = 512       # tokens per matmul (one PSUM bank of f32)
SUP = 2048     # tokens per super-tile (2 PSUM tiles of 1024)

# x-load chunk sizes in tokens (sum = MPC). Small head for early compute,
# 2 MiB bodies for DMA efficiency, small tail for a short drain.
LOAD_CHUNKS = [1024, 2048] + [4096] * 6 + [2048, 2048, 512, 512]
# y-store chunk sizes in tokens (sum = MPC); small tail shortens the final
# copy->store dependency chain.
STORE_CHUNKS = [8192] * 3 + [4096, 2048] + [1024, 1024]

assert sum(LOAD_CHUNKS) == MPC and sum(STORE_CHUNKS) == MPC


def _split_multi_waits(nc, mybir):
    """This walrus build rejects any instruction carrying more than one sync
    wait ("Too many sync wait commands", setupSyncWait). Hoist all but one
    wait of each offender onto fresh single-wait EventSemaphore instructions
    placed just before it on the same engine queue — semantically identical
    (sequencer-level blocking, monotonic sem conditions)."""
    for fn in nc.m.functions:
        for blk in fn.blocks:
            new_insts = []
            for inst in blk.instructions:
                si = getattr(inst, "sync_info", None)
                waits = list(si.on_wait) if si is not None and si.on_wait else []
                if len(waits) > 1:
                    for w in waits[:-1]:
                        name = nc.get_next_instruction_name()
                        ev = mybir.InstEventSemaphore(
                            name=name,
                            engine=inst.engine,
                            ins=[],
                            outs=[],
                            sync_info=mybir.SyncInfo(on_wait=[w], on_update=[]),
                        )
                        nc.inst_map[name] = ev
                        new_insts.append(ev)
                    si.on_wait = waits[-1:]
                new_insts.append(inst)
            blk.instructions = new_insts


def _build_bass():
    import concourse.bass as bass
    import concourse.mybir as mybir
    import concourse.tile as tile

    bf16 = mybir.dt.bfloat16
    f32 = mybir.dt.float32

    nc = bass.Bass()
    xT = nc.declare_dram_parameter("xT", [K, MPC], bf16, isOutput=False)
    wT = nc.declare_dram_parameter("wT", [K, N], bf16, isOutput=False)
    yT = nc.declare_dram_parameter("yT", [N, MPC], bf16, isOutput=True)

    with tile.TileContext(nc) as tc:
        with (
            tc.tile_pool(name="w", bufs=1) as wpool,
            tc.tile_pool(name="x", bufs=1) as xpool,
            tc.tile_pool(name="y", bufs=1) as ypool,
            tc.tile_pool(name="ps", bufs=2, space=bass.MemorySpace.PSUM) as pspool,
        ):
            # Weights on the ACT ring: lands in ~2 us, well before the first
            # matmul needs them; keeps the SP ring free for x loads.
            w_t = wpool.tile([K // 2, 2, N], bf16)
            nc.scalar.dma_start(
                w_t[:], wT[:, :].rearrange("(two p) n -> p two n", two=2)
            )

            # Whole x shard resident in SBUF; all loads issued immediately,
            # zero upstream dependencies.
            x_t = xpool.tile([K // 2, 2, MPC], bf16)
            off = 0
            for ch in LOAD_CHUNKS:
                nc.sync.dma_start(
                    x_t[:, :, off : off + ch],
                    xT[:, off : off + ch].rearrange("(two p) m -> p two m", two=2),
                )
                off += ch

            # Whole y shard resident in SBUF.
            y_t = ypool.tile([N, MPC], bf16)

            store_off = 0
            store_idx = 0
            next_store_at = STORE_CHUNKS[0]

            n_sup = MPC // SUP
            for si in range(n_sup - 1):
                s0 = si * SUP
                psa = pspool.tile([N, 1024], f32)
                psb = pspool.tile([N, 1024], f32)
                # Weight-phase batching: 4 matmuls per LDWEIGHTS.
                for ki, stop in ((0, False), (1, True)):
                    for j in range(4):
                        ms = s0 + j * PT
                        ps = psa if j < 2 else psb
                        o = (j % 2) * PT
                        nc.tensor.matmul(
                            ps[:, o : o + PT],
                            w_t[:, ki, :],
                            x_t[:, ki, ms : ms + PT],
                            start=(not stop),
                            stop=stop,
                        )
                # Cast copies: alternate DVE / ACT (GPSIMD cannot read PSUM).
                nc.vector.tensor_copy(y_t[:, s0 : s0 + 1024], psa[:])
                nc.scalar.copy(y_t[:, s0 + 1024 : s0 + 2048], psb[:])

                # Issue any store whose range is fully computed.
                while store_idx < len(STORE_CHUNKS) and s0 + SUP >= next_store_at:
                    ch = STORE_CHUNKS[store_idx]
                    nc.scalar.dma_start(
                        yT[:, store_off : store_off + ch],
                        y_t[:, store_off : store_off + ch],
                    )
                    store_off += ch
                    store_idx += 1
                    if store_idx < len(STORE_CHUNKS):
                        next_store_at = store_off + STORE_CHUNKS[store_idx]

            # Final super-tile at 512-token granularity: four independent
            # psum->copy->store chains (copies split DVE/ACT) so the drain
            # after the last load is as short as possible.
            s0 = (n_sup - 1) * SUP
            psa = pspool.tile([N, 1024], f32)
            psb = pspool.tile([N, 1024], f32)
            for ki, stop in ((0, False), (1, True)):
                for j in range(4):
                    ms = s0 + j * PT
                    ps = psa if j < 2 else psb
                    o = (j % 2) * PT
                    nc.tensor.matmul(
                        ps[:, o : o + PT],
                        w_t[:, ki, :],
                        x_t[:, ki, ms : ms + PT],
                        start=(not stop),
                        stop=stop,
                    )
            # Both tail copies on DVE (1024 wide) so the ACT sequencer only
            # generates the two store descriptors, each pipelined behind its
            # copy. One ~1 us DIRECT2D gen per store makes finer chunks a
            # net loss here.
            nc.vector.tensor_copy(y_t[:, s0 : s0 + 1024], psa[:])
            nc.scalar.dma_start(yT[:, s0 : s0 + 1024], y_t[:, s0 : s0 + 1024])
            nc.vector.tensor_copy(y_t[:, s0 + 1024 : s0 + 2048], psb[:])
            nc.scalar.dma_start(
                yT[:, s0 + 1024 : s0 + 2048], y_t[:, s0 + 1024 : s0 + 2048]
            )

    _split_multi_waits(nc, mybir)
    return nc


_NC_CACHE = None


def _get_nc():
    global _NC_CACHE
    if _NC_CACHE is None:
        _NC_CACHE = _build_bass()
    return _NC_CACHE


def _run(in_maps, **kwargs):
    from concourse.bass_utils import run_bass_kernel_spmd

    return run_bass_kernel_spmd(_get_nc(), in_maps, list(range(G)), **kwargs)


def make_in_maps(x, w):
    x = np.asarray(x)
    w = np.asarray(w)
    in_maps = []
    for g in range(G):
        xg = x[g * MPC : (g + 1) * MPC, :]
        wg = w[g * N : (g + 1) * N, :]
        in_maps.append(
            {
                "xT": np.ascontiguousarray(xg.T),
                "wT": np.ascontiguousarray(wg.T),
            }
        )
    return in_maps


def assemble(results, dtype):
    out = np.zeros((M, G * N), dtype=dtype)
    for g in range(G):
        yTg = np.asarray(results[g]["yT"])
        out[g * MPC : (g + 1) * MPC, g * N : (g + 1) * N] = yTg.T
    return out


def kernel(x, w):
    x = np.asarray(x)
    w = np.asarray(w)
    res = _run(make_in_maps(x, w))
    return assemble(res.results, x.dtype)
